# revision 1
# baseline (speedup 1.0000x reference)
"""DeepseekMoE kernel for 8 Trainium2 NeuronCores.

Strategy (expert-parallel + data-parallel shared experts):
  - Host computes the router (gate matmul, softmax, top-2) in numpy and
    gathers each expert's tokens (classic MoE dispatch, done host-side as
    part of sharding).
  - Core c runs routed expert c's FFN over its gathered tokens (padded to
    a common Cpad so all 8 cores run the same SPMD program), scaling the
    output by the combine weights on-device (DVE).
  - Shared experts' weights are replicated; each core runs them over a
    distinct 512-token slice of the batch (data-parallel).
  - All matmuls run in fp16 (1 cycle/row on the PE, same rate as bf16
    but with 10-bit mantissa -> ~8x less rounding error; fp32 is 4x
    slower) with fp32 PSUM accumulation; GELU (exact/erf) on the ACT
    engine reading PSUM directly.
  - Layout is fully transposed (features on partitions, tokens on the
    free dim) so the two FFN matmuls chain with no on-chip transposes.
    Host pre-packs every operand into [128, *] row-major blocks so each
    DMA is a contiguous >=512 KB transfer (HWDGE generation overhead is
    ~625 ns/DMA, so small DMAs cap effective HBM bandwidth).
  - The f-loop is software-pipelined (lookahead 2) across chunks and
    phases so the PE never stalls on ACT; output DMAs ride the SWDGE
    (gpsimd) path so they are not head-of-line blocked behind the
    input preload on the HWDGE queues.
  - Host scatters per-expert outputs back (each token appears in exactly
    K=2 experts) and adds the (zero, but handled exactly) output biases.
"""

import numpy as np
import ml_dtypes

import concourse.bass as bass
import concourse.tile as tile
import concourse.mybir as mybir
from concourse import bacc
from concourse.bass_utils import run_bass_kernel_spmd

B, S, D, F, E, NS, K = 2, 2048, 512, 2048, 8, 2, 2
T = B * S
N_CORES = 8
TS = T // N_CORES          # shared-expert tokens per core
FS = NS * F                # concatenated shared FFN width
CHUNK = 512                # token chunk (= max fp32 PSUM bank free dim)
KD = D // 128              # 4  k-tiles over D
FR = F // 128              # 16 f-tiles routed
FShared = FS // 128        # 32 f-tiles shared
DD = D // 128              # 4  output d-tiles
WG = 4                     # f-tiles per w-DMA group (512 KB transfers)

BF16 = mybir.dt.float16
F32 = mybir.dt.float32
np_bf16 = np.float16

_GELU = mybir.ActivationFunctionType.Gelu

_cache: dict = {}


def _routed_sizes(cpad):
    """Token-chunk sizes for the routed phase: a small first chunk (fast PE
    start — less DMA to wait for), 512s in the middle, and a smallish final
    chunk (short drain tail). No chunk below 256 — small-N matmuls go
    LDWEIGHTS-bound on real hardware."""
    if cpad <= CHUNK:
        return [cpad]
    head = cpad - 768
    if 256 <= head <= CHUNK:                # the realistic range
        return [head, CHUNK, 256]
    if cpad < 1024:
        mid = cpad - 512
        return [256] + ([mid] if mid else []) + [256]
    sizes, rem = [256], cpad - 768          # reserve two 256 tail chunks
    while rem > CHUNK:
        take = CHUNK if rem - CHUNK >= 256 else rem - 256
        sizes.append(take)
        rem -= take
    sizes.append(rem)
    return sizes + [256, 256]


def _shared_sizes(ts):
    """Shared-expert chunk sizes; ends on a 256 chunk for a short tail."""
    return [ts] if ts <= 256 else [ts - 256, 256]


def _chunk_offsets(total, sizes=None):
    """(start, size) pairs; default uniform CHUNK split."""
    if sizes is None:
        sizes = [min(CHUNK, total - c0) for c0 in range(0, total, CHUNK)]
    out, c0 = [], 0
    for s in sizes:
        out.append((c0, s))
        c0 += s
    return out


def _build(cpad: int):
    nc = bacc.Bacc("TRN2", debug=False)

    xg = nc.dram_tensor("xg", [128, KD * cpad], BF16, kind="ExternalInput")
    cwb = nc.dram_tensor("cwb", [128, cpad], F32, kind="ExternalInput")
    rw1t = nc.dram_tensor("rw1t", [128, KD * F], BF16, kind="ExternalInput")
    rw2t = nc.dram_tensor("rw2t", [128, FR * D], BF16, kind="ExternalInput")
    rb1 = nc.dram_tensor("rb1", [128, FR], F32, kind="ExternalInput")
    xs = nc.dram_tensor("xs", [128, KD * TS], BF16, kind="ExternalInput")
    sw1t = nc.dram_tensor("sw1t", [128, KD * FS], BF16, kind="ExternalInput")
    sw2t = nc.dram_tensor("sw2t", [128, FShared * D], BF16, kind="ExternalInput")
    sb1 = nc.dram_tensor("sb1", [128, FShared], F32, kind="ExternalInput")
    yr = nc.dram_tensor("yr", [D, cpad], F32, kind="ExternalOutput")
    ys = nc.dram_tensor("ys", [D, TS], BF16, kind="ExternalOutput")

    with tile.TileContext(nc) as tc:
        with (
            tc.tile_pool(name="wts", bufs=1) as wts,
            tc.tile_pool(name="acts", bufs=1) as acts,
            tc.tile_pool(name="hp", bufs=4) as hp,
            tc.tile_pool(name="op", bufs=3) as op,
            tc.tile_pool(name="ps1", bufs=4, space="PSUM") as ps1,
            tc.tile_pool(name="ps2", bufs=1, space="PSUM") as ps2,
        ):
            # ---- t=0 warmup while the first DMAs are in flight: trigger the
            # GELU ACT-table load now (it costs ~1.3 us on first use), and run
            # dummy matmuls so the PE p-state/HAM is at full clock when the
            # first real matmul issues ----
            warm = wts.tile([128, 512], BF16, name="warm_in")
            nc.vector.memset(warm[:], 0.0)
            wb = wts.tile([128, 1], F32, name="warm_b")
            nc.vector.memset(wb[:], 0.0)
            wh = hp.tile([128, 512], BF16, name="wh")
            nc.scalar.activation(wh[:], warm[:, 0:512], _GELU, bias=wb[:])
            wp = ps1.tile([128, 512], F32, tag="p1", name="warmp")
            for _ in range(6):
                nc.tensor.matmul(wp[:], warm[:, 0:128], warm[:], start=True, stop=True)

            # ---- resident SBUF images of all inputs ----
            xg_sb = acts.tile([128, KD * cpad], BF16, name="xg_sb")
            rw1_sb = wts.tile([128, KD * F], BF16, name="rw1_sb")
            rw2_sb = wts.tile([128, FR * D], BF16, name="rw2_sb")
            rb1_sb = wts.tile([128, FR], F32, name="rb1_sb")
            cw_sb = acts.tile([128, cpad], F32, name="cw_sb")
            xs_sb = acts.tile([128, KD * TS], BF16, name="xs_sb")
            sw1_sb = wts.tile([128, KD * FS], BF16, name="sw1_sb")
            sw2_sb = wts.tile([128, FShared * D], BF16, name="sw2_sb")
            sb1_sb = wts.tile([128, FShared], F32, name="sb1_sb")

            def col_dma(dst, src, lo, hi):
                nc.sync.dma_start(dst[:, lo:hi], src.ap()[:, lo:hi])

            def w1_group_dma(dst, src, f_lo, f_hi):
                # f-columns [f_lo*128, f_hi*128) for every k-block
                d4 = dst.rearrange("p (k f) -> p k f", k=KD)
                s4 = src.ap().rearrange("p (k f) -> p k f", k=KD)
                nc.sync.dma_start(d4[:, :, f_lo * 128:f_hi * 128],
                                  s4[:, :, f_lo * 128:f_hi * 128])

            # consumption-ordered preload (HWDGE)
            chunks_r = _chunk_offsets(cpad, _routed_sizes(cpad))
            c0, cs = chunks_r[0]
            xoff = [0]
            for _, s in chunks_r:
                xoff.append(xoff[-1] + KD * s)
            # chunk-0 tokens ride SWDGE so their descriptor generation runs in
            # parallel with rw1's HWDGE generation (shorter startup chain)
            nc.gpsimd.dma_start(xg_sb[:, 0:xoff[1]], xg.ap()[:, 0:xoff[1]])
            w1_group_dma(rw1_sb, rw1t, 0, 2)                    # rw1 f0..f1
            nc.sync.dma_start(rb1_sb[:], rb1.ap())
            col_dma(rw2_sb, rw2t, 0, WG * D)                    # rw2 f0..f3
            w1_group_dma(rw1_sb, rw1t, 2, 4)
            for g in range(1, FR // WG):
                w1_group_dma(rw1_sb, rw1t, g * WG, (g + 1) * WG)
                col_dma(rw2_sb, rw2t, g * WG * D, (g + 1) * WG * D)
            col_dma(xg_sb, xg, xoff[1], xoff[-1])               # remaining tokens
            nc.sync.dma_start(cw_sb[:], cwb.ap())
            nc.sync.dma_start(xs_sb[:], xs.ap())
            nc.sync.dma_start(sb1_sb[:], sb1.ap())
            for g in range(FShared // (2 * WG)):                # 1 MB transfers
                w1_group_dma(sw1_sb, sw1t, g * 2 * WG, (g + 1) * 2 * WG)
                col_dma(sw2_sb, sw2t, g * 2 * WG * D, (g + 1) * 2 * WG * D)

            # ---- chunk descriptors: small routed chunk first (fast start),
            # shared phase last, ending on a small chunk (short tail) ----
            def r_chunk(i, c0, cs):
                return dict(
                    cs=cs, c0=c0, nf=FR, cw=True, y=yr, b1=rb1_sb,
                    x=lambda k, o=xoff[i], cs=cs: xg_sb[:, o + k * cs:o + (k + 1) * cs],
                    w1=lambda k, f: rw1_sb[:, k * F + f * 128:k * F + (f + 1) * 128],
                    w2=lambda f, d: rw2_sb[:, f * D + d * 128:f * D + (d + 1) * 128],
                )

            def s_chunk(i, c0, cs):
                return dict(
                    cs=cs, c0=c0, nf=FShared, cw=False, y=ys, b1=sb1_sb,
                    x=lambda k, i=i, cs=cs: xs_sb[:, soff[i] + k * cs:soff[i] + (k + 1) * cs],
                    w1=lambda k, f: sw1_sb[:, k * FS + f * 128:k * FS + (f + 1) * 128],
                    w2=lambda f, d: sw2_sb[:, f * D + d * 128:f * D + (d + 1) * 128],
                )

            chunks_s = _chunk_offsets(TS, _shared_sizes(TS))
            soff = [0]
            for _, s in chunks_s:
                soff.append(soff[-1] + KD * s)
            routed = [r_chunk(i, c0, cs) for i, (c0, cs) in enumerate(chunks_r)]
            shared = [s_chunk(i, c0, cs) for i, (c0, cs) in enumerate(chunks_s)]
            chunks = routed + shared
            steps = [(ch, f) for ch in chunks for f in range(ch["nf"])]

            # ---- software-pipelined emission: PE issues the f-tile's
            # first-layer matmuls LOOKAHEAD steps ahead of the second-layer
            # matmuls that consume the GELU output ----
            LOOKAHEAD = 2
            h_tiles: dict = {}
            po_tiles: dict = {}
            for i in range(len(steps) + LOOKAHEAD):
                if i < len(steps):
                    ch, f = steps[i]
                    cs = ch["cs"]
                    p1 = ps1.tile([128, cs], F32, name="p1")
                    for k in range(KD):
                        nc.tensor.matmul(
                            p1[:], ch["w1"](k, f), ch["x"](k),
                            start=(k == 0), stop=(k == KD - 1),
                        )
                    h = hp.tile([128, cs], BF16, name="h")
                    nc.scalar.activation(h[:], p1[:], _GELU, bias=ch["b1"][:, f:f + 1])
                    h_tiles[i] = h
                j = i - LOOKAHEAD
                if j >= 0:
                    ch, f = steps[j]
                    cs, c0 = ch["cs"], ch["c0"]
                    if f == 0:
                        po_tiles[id(ch)] = [
                            ps2.tile([128, cs], F32, tag=f"o{d}", name=f"po{d}")
                            for d in range(DD)
                        ]
                    po = po_tiles[id(ch)]
                    h = h_tiles.pop(j)
                    for d in range(DD):
                        nc.tensor.matmul(
                            po[d][:], ch["w2"](f, d), h[:],
                            start=(f == 0), stop=(f == ch["nf"] - 1),
                        )
                    if f == ch["nf"] - 1:
                        o = op.tile([128, DD * cs], F32 if ch["cw"] else BF16,
                                    name="o")
                        last = ch is chunks[-1]
                        for d in range(DD):
                            if ch["cw"]:
                                nc.vector.tensor_mul(
                                    o[:, d * cs:(d + 1) * cs], po[d][:],
                                    cw_sb[:, c0:c0 + cs])
                            elif last and d >= 2:
                                # tail chunk: split evacuation across ACT and
                                # DVE so the final drain starts sooner
                                nc.scalar.copy(o[:, d * cs:(d + 1) * cs], po[d][:])
                            else:
                                nc.vector.tensor_copy(
                                    o[:, d * cs:(d + 1) * cs], po[d][:])
                        # one wide DMA per chunk on the SWDGE path: separate
                        # FIFO from the input preload (no head-of-line block),
                        # and one generation overhead instead of four. The
                        # final chunk rides HWDGE (lower latency; preload is
                        # long finished) to shorten the kernel tail.
                        ydst = ch["y"].ap().rearrange(
                            "(dd p) c -> p dd c", p=128)[:, :, c0:c0 + cs]
                        ysrc = o.rearrange("p (dd c) -> p dd c", dd=DD)
                        if last:
                            nc.sync.dma_start(ydst, ysrc)
                        else:
                            nc.gpsimd.dma_start(ydst, ysrc)
                        del po_tiles[id(ch)]

    nc.compile()
    return nc


def _pack_k_blocks(a2d):
    """[K*128, N] -> [128, K*N] with k-blocks along the free dim."""
    k = a2d.shape[0] // 128
    return np.ascontiguousarray(
        a2d.reshape(k, 128, -1).transpose(1, 0, 2).reshape(128, -1))


def _pack_chunked(xT, total, sizes=None):
    """[D, total] -> [128, KD*total] grouped chunk-major: for each chunk c,
    the KD k-blocks of that chunk's columns are laid out consecutively."""
    parts = []
    for c0, cs in _chunk_offsets(total, sizes):
        blk = xT[:, c0:c0 + cs]                      # [D, cs]
        parts.append(blk.reshape(KD, 128, cs).transpose(1, 0, 2).reshape(128, -1))
    return np.ascontiguousarray(np.concatenate(parts, axis=1))


def kernel(x, gate_w, gate_b, sw1, sb1, sw2, sb2, rw1, rb1, rw2, rb2):
    x = np.asarray(x, np.float32)
    gate_w = np.asarray(gate_w, np.float32)
    gate_b = np.asarray(gate_b, np.float32)
    sw1 = np.asarray(sw1, np.float32)
    sb1 = np.asarray(sb1, np.float32)
    sw2 = np.asarray(sw2, np.float32)
    sb2 = np.asarray(sb2, np.float32)
    rw1 = np.asarray(rw1, np.float32)
    rb1 = np.asarray(rb1, np.float32)
    rw2 = np.asarray(rw2, np.float32)
    rb2 = np.asarray(rb2, np.float32)

    t = x.reshape(T, D)

    # ---- router on host (part of the dispatch/sharding step) ----
    logits = t @ gate_w.T + gate_b
    m = logits.max(axis=1, keepdims=True)
    ex = np.exp(logits - m)
    probs = ex / ex.sum(axis=1, keepdims=True)
    top_i = np.argpartition(-probs, K - 1, axis=1)[:, :K]          # [T, K]

    sel = np.zeros((T, E), bool)
    sel[np.arange(T)[:, None], top_i] = True
    idxs = [np.nonzero(sel[:, e])[0] for e in range(E)]
    counts = np.array([len(i) for i in idxs])
    cpad = max(CHUNK, int(-(-counts.max() // 4) * 4))

    if cpad not in _cache:
        _cache[cpad] = _build(cpad)
    nc = _cache[cpad]

    # ---- shared-expert weights, concatenated over NS and packed ----
    sw1t = _pack_k_blocks(sw1.reshape(FS, D).T.astype(np_bf16))
    sw2t = _pack_k_blocks(sw2.transpose(0, 2, 1).reshape(FS, D).astype(np_bf16))
    sb1c = np.ascontiguousarray(sb1.reshape(FShared, 128).T)

    in_maps = []
    for c in range(N_CORES):
        idx = idxs[c]
        ce = len(idx)
        xgT = np.zeros((D, cpad), np_bf16)
        xgT[:, :ce] = t[idx].T.astype(np_bf16)
        cwb = np.zeros((128, cpad), np.float32)
        cwb[:, :ce] = probs[idx, c][None, :]
        in_maps.append({
            "xg": _pack_chunked(xgT, cpad, _routed_sizes(cpad)),
            "cwb": cwb,
            "rw1t": _pack_k_blocks(rw1[c].T.astype(np_bf16)),
            "rw2t": _pack_k_blocks(rw2[c].T.astype(np_bf16)),
            "rb1": np.ascontiguousarray(rb1[c].reshape(FR, 128).T),
            "xs": _pack_chunked(
                np.ascontiguousarray(t[c * TS:(c + 1) * TS].T.astype(np_bf16)),
                TS, _shared_sizes(TS)),
            "sw1t": sw1t,
            "sw2t": sw2t,
            "sb1": sb1c,
        })

    res = run_bass_kernel_spmd(nc, in_maps, core_ids=list(range(N_CORES)))

    # ---- combine on host ----
    out = np.empty((T, D), np.float32)
    for c in range(N_CORES):
        out[c * TS:(c + 1) * TS] = res.results[c]["ys"].T.astype(np.float32)
    for c in range(N_CORES):
        idx = idxs[c]
        out[idx] += res.results[c]["yr"][:, :len(idx)].T

    # output biases (zero in the spec, handled exactly anyway)
    if sb2.any() or rb2.any():
        cw = np.zeros((T, E), np.float32)
        np.add.at(cw, (np.arange(T)[:, None], top_i),
                  np.take_along_axis(probs, top_i, axis=1))
        out += sb2.sum(axis=0)[None, :] + cw @ rb2

    return out.reshape(B, S, D)



# revision 5
# speedup vs baseline: 1.4956x; 1.4956x over previous
"""DeepseekMoE kernel for 8 Trainium2 NeuronCores.

Strategy (expert-parallel routed + data-parallel shared, fp8 DoubleRow):
  - Host computes the router (gate matmul, softmax, top-2) in numpy and
    gathers each expert's tokens (MoE dispatch as part of sharding).
  - Core c runs routed expert c's FFN over its gathered tokens; shared
    experts are replicated and each core runs them over a distinct
    512-token slice of the batch.
  - All heavy matmuls use fp8(e4m3) in DoubleRow perf mode: each
    instruction contracts 2 k-tiles (256 rows) at 0.5 cycles per output
    column -- 4x the fp16 rate on the PE.
  - Accuracy: weights are pre-scaled by 2^11 so their hi/lo fp8 splits
    stay out of e4m3's subnormal floor; the descale folds into the ACT
    input scale (layer 1) and the host-side combine weights (layer 2).
      * shared experts (~98% of output norm): layer 1 runs a 3-matmul
        hi/lo compensation (w_hi*x_hi + w_lo*x_hi + w_hi*x_lo), h is
        kept in fp16 and layer 2 runs as a plain fp16 matmul.
      * routed experts (~21% of output norm): plain fp8 both layers.
    Measured end-to-end rel err ~1.1% vs the 2e-2 gate.
  - Layer 2 keeps tokens on the output partition dim (stationary = h,
    moving = w2), so each 128-token block accumulates into exactly one
    PSUM bank, the combine weight is a per-partition tensor_scalar
    multiply, and outputs land in [token, D] layout (no host transpose).
  - Routed and shared f-tile steps are interleaved (Bresenham weave) so
    the ACT-heavy routed phase overlaps the PE-heavy shared phase, with
    a 2-step software pipeline between layer 1 and layer 2.
  - 256-token chunks keep both phases' PSUM footprint at 8 banks total
    (2x p1 double-buffer + 2 token-block accumulators per phase).
"""

import numpy as np
import ml_dtypes

import concourse.bass as bass
import concourse.tile as tile
import concourse.mybir as mybir
from concourse import bacc
from concourse.bass_utils import run_bass_kernel_spmd

B, S, D, F, E, NS, K = 2, 2048, 512, 2048, 8, 2, 2
T = B * S
N_CORES = 8
TS = T // N_CORES          # shared-expert tokens per core
FS = NS * F                # concatenated shared FFN width
CS = 256                   # token chunk (1 PSUM bank per 2-f-tile p1 pair)
KD = D // 128              # 4 k-tiles over D
FR = F // 128              # 16 f-tiles routed
FSH = FS // 128            # 32 f-tiles shared
PR = FR // 2               # 8 routed f-pairs
PS = FSH // 2              # 16 shared f-pairs
WS = 2048.0                # 2^11 weight pre-scale (exact power of two)

F8 = mybir.dt.float8e4
F16 = mybir.dt.float16
F32 = mybir.dt.float32
np8 = ml_dtypes.float8_e4m3
np16 = np.float16

_GELU = mybir.ActivationFunctionType.Gelu
_DR = mybir.MatmulPerfMode.DoubleRow

_cache: dict = {}


def _chunks(total):
    """(start, size) chunk list: CS-sized chunks plus a ragged tail."""
    out, c0 = [], 0
    while c0 < total:
        cs = min(CS, total - c0)
        out.append((c0, cs))
        c0 += cs
    return out


def _weave(ra, sb):
    """Evenly interleave two lists (Bresenham pacing)."""
    out, i, j = [], 0, 0
    while i < len(ra) or j < len(sb):
        if j >= len(sb) or (i < len(ra) and i * len(sb) <= j * len(ra)):
            out.append(ra[i]); i += 1
        else:
            out.append(sb[j]); j += 1
    return out


def _build(cpad: int):
    nc = bacc.Bacc("TRN2", debug=False)

    rchunks = _chunks(cpad)
    schunks = _chunks(TS)
    ntb_r = sum(-(-cs // 128) for _, cs in rchunks)

    xg = nc.dram_tensor("xg", [128, KD * cpad], F8, kind="ExternalInput")
    cwp = nc.dram_tensor("cwp", [128, ntb_r], F32, kind="ExternalInput")
    rw1 = nc.dram_tensor("rw1", [128, KD * F], F8, kind="ExternalInput")
    rw2 = nc.dram_tensor("rw2", [128, F * KD], F8, kind="ExternalInput")
    xsh = nc.dram_tensor("xsh", [128, KD * TS], F8, kind="ExternalInput")
    xsl = nc.dram_tensor("xsl", [128, KD * TS], F8, kind="ExternalInput")
    sw1h = nc.dram_tensor("sw1h", [128, KD * FS], F8, kind="ExternalInput")
    sw1l = nc.dram_tensor("sw1l", [128, KD * FS], F8, kind="ExternalInput")
    sw2 = nc.dram_tensor("sw2", [128, FSH * D], F16, kind="ExternalInput")
    yr = nc.dram_tensor("yr", [cpad, D], F16, kind="ExternalOutput")
    ys = nc.dram_tensor("ys", [TS, D], F16, kind="ExternalOutput")

    with tile.TileContext(nc) as tc:
        with (
            tc.tile_pool(name="wts", bufs=1) as wts,
            tc.tile_pool(name="acts", bufs=1) as acts,
            tc.tile_pool(name="hpr", bufs=4) as hpr,
            tc.tile_pool(name="hps", bufs=4) as hps,
            tc.tile_pool(name="op", bufs=3) as op,
            tc.tile_pool(name="psr", bufs=2, space="PSUM") as psr,
            tc.tile_pool(name="pss", bufs=2, space="PSUM") as pss,
            tc.tile_pool(name="por", bufs=1, space="PSUM") as por,
            tc.tile_pool(name="pos", bufs=1, space="PSUM") as pos,
        ):
            # ---- warmup while the first DMAs are in flight: trigger the
            # GELU table load and ramp the PE p-state ----
            warm = wts.tile([128, 512], F8, name="warm_in")
            nc.vector.memset(warm[:], 0.0)
            wh = hpr.tile([128, 512], F8, name="warm_h")
            nc.scalar.activation(wh[:], warm[:, 0:512], _GELU, bias=0.0)
            wp = psr.tile([128, 512], F32, tag="p1", name="warm_p")
            w3 = warm.rearrange("p (two n) -> p two n", two=2)
            for _ in range(5):
                nc.tensor.matmul(wp[:, 0:256], w3[:, :, 0:128], w3[:],
                                 start=True, stop=True, perf_mode=_DR)

            # ---- resident SBUF images ----
            xg_sb = acts.tile([128, KD * cpad], F8, name="xg_sb")
            cwp_sb = acts.tile([128, ntb_r], F32, name="cwp_sb")
            rw1_sb = wts.tile([128, KD * F], F8, name="rw1_sb")
            rw2_sb = wts.tile([128, F * KD], F8, name="rw2_sb")
            xsh_sb = acts.tile([128, KD * TS], F8, name="xsh_sb")
            xsl_sb = acts.tile([128, KD * TS], F8, name="xsl_sb")
            sw1h_sb = wts.tile([128, KD * FS], F8, name="sw1h_sb")
            sw1l_sb = wts.tile([128, KD * FS], F8, name="sw1l_sb")
            sw2_sb = wts.tile([128, FSH * D], F16, name="sw2_sb")

            # 3-d views for DoubleRow operand slicing
            rw1_v = rw1_sb.rearrange("p (k f) -> p k f", k=KD)
            rw2_v = rw2_sb.rearrange("p (j two d) -> p j two d", j=PR, two=2)
            sw1h_v = sw1h_sb.rearrange("p (k f) -> p k f", k=KD)
            sw1l_v = sw1l_sb.rearrange("p (k f) -> p k f", k=KD)
            sw2_v = sw2_sb.rearrange("p (f d) -> p f d", f=FSH)
            xg_v = xg_sb  # chunk-major; sliced via offsets below
            xoff = [0]
            for _, cs in rchunks:
                xoff.append(xoff[-1] + KD * cs)
            soff = [0]
            for _, cs in schunks:
                soff.append(soff[-1] + KD * cs)

            def w1_group_dma(dst, src, f_lo, f_hi, n_f):
                d3 = dst.rearrange("p (k f) -> p k f", k=KD)
                s3 = src.ap().rearrange("p (k f) -> p k f", k=KD)
                nc.sync.dma_start(d3[:, :, f_lo * 128:f_hi * 128],
                                  s3[:, :, f_lo * 128:f_hi * 128])

            # ---- consumption-ordered preload ----
            nc.gpsimd.dma_start(xg_sb[:, 0:xoff[1]], xg.ap()[:, 0:xoff[1]])
            w1_group_dma(rw1_sb, rw1, 0, 4, FR)
            nc.gpsimd.dma_start(xsh_sb[:], xsh.ap())
            nc.gpsimd.dma_start(xsl_sb[:], xsl.ap())
            w1_group_dma(sw1h_sb, sw1h, 0, 8, FSH)
            w1_group_dma(sw1l_sb, sw1l, 0, 8, FSH)
            nc.sync.dma_start(rw2_sb[:, 0:4 * 2 * D], rw2.ap()[:, 0:4 * 2 * D])
            nc.sync.dma_start(sw2_sb[:, 0:8 * D], sw2.ap()[:, 0:8 * D])
            w1_group_dma(rw1_sb, rw1, 4, 8, FR)
            if len(rchunks) > 1:
                nc.gpsimd.dma_start(xg_sb[:, xoff[1]:xoff[2]],
                                    xg.ap()[:, xoff[1]:xoff[2]])
            w1_group_dma(sw1h_sb, sw1h, 8, 16, FSH)
            w1_group_dma(sw1l_sb, sw1l, 8, 16, FSH)
            nc.sync.dma_start(rw2_sb[:, 4 * 2 * D:], rw2.ap()[:, 4 * 2 * D:])
            nc.sync.dma_start(sw2_sb[:, 8 * D:16 * D], sw2.ap()[:, 8 * D:16 * D])
            w1_group_dma(rw1_sb, rw1, 8, 12, FR)
            nc.sync.dma_start(cwp_sb[:], cwp.ap())
            if len(rchunks) > 2:
                nc.gpsimd.dma_start(xg_sb[:, xoff[2]:xoff[3]],
                                    xg.ap()[:, xoff[2]:xoff[3]])
            w1_group_dma(sw1h_sb, sw1h, 16, 24, FSH)
            w1_group_dma(sw1l_sb, sw1l, 16, 24, FSH)
            nc.sync.dma_start(sw2_sb[:, 16 * D:24 * D], sw2.ap()[:, 16 * D:24 * D])
            w1_group_dma(rw1_sb, rw1, 12, 16, FR)
            if len(rchunks) > 3:
                nc.gpsimd.dma_start(xg_sb[:, xoff[3]:], xg.ap()[:, xoff[3]:])
            w1_group_dma(sw1h_sb, sw1h, 24, 32, FSH)
            w1_group_dma(sw1l_sb, sw1l, 24, 32, FSH)
            nc.sync.dma_start(sw2_sb[:, 24 * D:], sw2.ap()[:, 24 * D:])

            # ---- step list: (phase, chunk_idx, pair_idx), woven ----
            rsteps = [("R", ci, j) for ci in range(len(rchunks))
                      for j in range(PR)]
            ssteps = [("S", ci, j) for ci in range(len(schunks))
                      for j in range(PS)]
            steps = _weave(rsteps, ssteps)

            # per-chunk global t-block column base for cwp
            tb_base = []
            acc = 0
            for _, cs in rchunks:
                tb_base.append(acc)
                acc += -(-cs // 128)

            h_tiles: dict = {}
            po_r: list = [None]
            po_s: list = [None]

            def stage_a(step):
                ph, ci, j = step
                if ph == "R":
                    c0, cs = rchunks[ci]
                    p1 = psr.tile([128, 2 * cs], F32, tag="p1", name="p1r")
                    for i01 in range(2):
                        f = 2 * j + i01
                        dst = p1[:, i01 * cs:(i01 + 1) * cs]
                        xo = xoff[ci]
                        for b in range(2):
                            nc.tensor.matmul(
                                dst,
                                rw1_v[:, 2 * b:2 * b + 2, f * 128:(f + 1) * 128],
                                xg_v[:, xo:xo + KD * cs].rearrange(
                                    "p (k c) -> p k c", k=KD)[:, 2 * b:2 * b + 2, :],
                                start=(b == 0), stop=(b == 1), perf_mode=_DR)
                    h = hpr.tile([128, 2 * cs], F8, name="hr")
                    nc.scalar.activation(h[:], p1[:], _GELU, bias=0.0,
                                         scale=1.0 / WS)
                else:
                    c0, cs = schunks[ci]
                    p1 = pss.tile([128, 2 * cs], F32, tag="p1", name="p1s")
                    xh3 = xsh_sb[:, soff[ci]:soff[ci] + KD * cs].rearrange(
                        "p (k c) -> p k c", k=KD)
                    xl3 = xsl_sb[:, soff[ci]:soff[ci] + KD * cs].rearrange(
                        "p (k c) -> p k c", k=KD)
                    for i01 in range(2):
                        f = 2 * j + i01
                        dst = p1[:, i01 * cs:(i01 + 1) * cs]
                        fs = slice(f * 128, (f + 1) * 128)
                        for b in range(2):
                            ks = slice(2 * b, 2 * b + 2)
                            nc.tensor.matmul(
                                dst, sw1h_v[:, ks, fs], xh3[:, ks, :],
                                start=(b == 0), stop=False, perf_mode=_DR)
                            nc.tensor.matmul(
                                dst, sw1l_v[:, ks, fs], xh3[:, ks, :],
                                start=False, stop=False, perf_mode=_DR)
                            nc.tensor.matmul(
                                dst, sw1h_v[:, ks, fs], xl3[:, ks, :],
                                start=False, stop=(b == 1), perf_mode=_DR)
                    h = hps.tile([128, 2 * cs], F16, name="hs")
                    nc.scalar.activation(h[:], p1[:], _GELU, bias=0.0,
                                         scale=1.0 / WS)
                return h

            def stage_b(step, h, is_last_step):
                ph, ci, j = step
                if ph == "R":
                    c0, cs = rchunks[ci]
                    ntb = -(-cs // 128)
                    if j == 0:
                        po_r[0] = [por.tile([128, D], F32, tag=f"r{tb}",
                                            name=f"por{tb}")
                                   for tb in range(ntb)]
                    h3 = h.rearrange("p (two c) -> p two c", two=2)
                    for tb in range(ntb):
                        tbs = min(128, cs - tb * 128)
                        nc.tensor.matmul(
                            po_r[0][tb][0:tbs, :],
                            h3[:, :, tb * 128:tb * 128 + tbs],
                            rw2_v[:, j, :, :],
                            start=(j == 0), stop=(j == PR - 1), perf_mode=_DR)
                    if j == PR - 1:
                        o = op.tile([128, ntb * D], F16, name="or")
                        for tb in range(ntb):
                            tbs = min(128, cs - tb * 128)
                            nc.vector.tensor_scalar_mul(
                                o[0:tbs, tb * D:(tb + 1) * D],
                                po_r[0][tb][0:tbs, :],
                                cwp_sb[0:tbs, tb_base[ci] + tb:
                                       tb_base[ci] + tb + 1])
                        if cs % 128 == 0:
                            dst = yr.ap()[c0:c0 + cs, :].rearrange(
                                "(tb p) d -> p tb d", p=128)
                            src = o.rearrange("p (tb d) -> p tb d", tb=ntb)
                            dma = nc.sync.dma_start if is_last_step \
                                else nc.gpsimd.dma_start
                            dma(dst, src)
                        else:
                            dma = nc.sync.dma_start if is_last_step \
                                else nc.gpsimd.dma_start
                            dma(yr.ap()[c0:c0 + cs, :], o[0:cs, 0:D])
                else:
                    c0, cs = schunks[ci]
                    ntb = -(-cs // 128)
                    if j == 0:
                        po_s[0] = [pos.tile([128, D], F32, tag=f"s{tb}",
                                            name=f"pos{tb}")
                                   for tb in range(ntb)]
                    for i01 in range(2):
                        f = 2 * j + i01
                        for tb in range(ntb):
                            tbs = min(128, cs - tb * 128)
                            nc.tensor.matmul(
                                po_s[0][tb][0:tbs, :],
                                h[:, i01 * cs + tb * 128:
                                  i01 * cs + tb * 128 + tbs],
                                sw2_v[:, f, :],
                                start=(f == 0), stop=(f == FSH - 1))
                    if j == PS - 1:
                        o = op.tile([128, ntb * D], F16, name="os")
                        for tb in range(ntb):
                            tbs = min(128, cs - tb * 128)
                            nc.vector.tensor_copy(
                                o[0:tbs, tb * D:(tb + 1) * D],
                                po_s[0][tb][0:tbs, :])
                        dst = ys.ap()[c0:c0 + cs, :].rearrange(
                            "(tb p) d -> p tb d", p=128)
                        src = o.rearrange("p (tb d) -> p tb d", tb=ntb)
                        dma = nc.sync.dma_start if is_last_step \
                            else nc.gpsimd.dma_start
                        dma(dst, src)

            LOOKAHEAD = 2
            for i in range(len(steps) + LOOKAHEAD):
                if i < len(steps):
                    h_tiles[i] = stage_a(steps[i])
                jj = i - LOOKAHEAD
                if jj >= 0:
                    stage_b(steps[jj], h_tiles.pop(jj), jj == len(steps) - 1)

    nc.compile()
    return nc


def _pack_k_blocks(a2d, dtype):
    """[K*128, N] -> [128, K*N] with k-blocks along the free dim."""
    k = a2d.shape[0] // 128
    return np.ascontiguousarray(
        a2d.reshape(k, 128, -1).transpose(1, 0, 2).reshape(128, -1)
        .astype(dtype))


def _pack_chunked(xT, total, dtype):
    """[D, total] -> [128, KD*total] chunk-major k-blocks."""
    parts = []
    for c0, cs in _chunks(total):
        blk = xT[:, c0:c0 + cs]
        parts.append(blk.reshape(KD, 128, cs).transpose(1, 0, 2)
                     .reshape(128, -1))
    return np.ascontiguousarray(np.concatenate(parts, axis=1).astype(dtype))


def _split8(a):
    """Scaled hi/lo e4m3 split of an array (applied at scale WS)."""
    hi = a.astype(np8)
    lo = (a - hi.astype(np.float32)).astype(np8)
    return hi, lo


def _numpy_fallback(x, gate_w, gate_b, sw1, sb1, sw2, sb2, rw1, rb1, rw2, rb2):
    from scipy.special import erf
    t = x.reshape(-1, D)
    gelu = lambda u: 0.5 * u * (1.0 + erf(u / np.sqrt(2.0)))
    hs = gelu(np.einsum('td,nfd->ntf', t, sw1) + sb1[:, None, :])
    shared = np.einsum('ntf,ndf->td', hs, sw2) + sb2.sum(axis=0)
    logits = t @ gate_w.T + gate_b
    m = logits.max(axis=1, keepdims=True)
    ex = np.exp(logits - m)
    probs = ex / ex.sum(axis=1, keepdims=True)
    top_i = np.argpartition(-probs, K - 1, axis=1)[:, :K]
    cw = np.zeros_like(probs)
    np.add.at(cw, (np.arange(t.shape[0])[:, None], top_i),
              np.take_along_axis(probs, top_i, axis=1))
    hr = gelu(np.einsum('td,efd->etf', t, rw1) + rb1[:, None, :])
    oe = np.einsum('etf,edf->etd', hr, rw2) + rb2[:, None, :]
    routed = np.einsum('etd,te->td', oe, cw)
    return (shared + routed).reshape(x.shape).astype(np.float32)


def kernel(x, gate_w, gate_b, sw1, sb1, sw2, sb2, rw1, rb1, rw2, rb2):
    x = np.asarray(x, np.float32)
    gate_w = np.asarray(gate_w, np.float32)
    gate_b = np.asarray(gate_b, np.float32)
    sw1 = np.asarray(sw1, np.float32)
    sb1 = np.asarray(sb1, np.float32)
    sw2 = np.asarray(sw2, np.float32)
    sb2 = np.asarray(sb2, np.float32)
    rw1 = np.asarray(rw1, np.float32)
    rb1 = np.asarray(rb1, np.float32)
    rw2 = np.asarray(rw2, np.float32)
    rb2 = np.asarray(rb2, np.float32)

    if sb1.any() or rb1.any():
        # device path folds first-layer biases away (they are zero in the
        # problem spec); fall back to exact numpy if that ever changes
        return _numpy_fallback(x, gate_w, gate_b, sw1, sb1, sw2, sb2,
                               rw1, rb1, rw2, rb2)

    t = x.reshape(T, D)

    # ---- router on host (dispatch/sharding step) ----
    logits = t @ gate_w.T + gate_b
    m = logits.max(axis=1, keepdims=True)
    ex = np.exp(logits - m)
    probs = ex / ex.sum(axis=1, keepdims=True)
    top_i = np.argpartition(-probs, K - 1, axis=1)[:, :K]

    sel = np.zeros((T, E), bool)
    sel[np.arange(T)[:, None], top_i] = True
    idxs = [np.nonzero(sel[:, e])[0] for e in range(E)]
    counts = np.array([len(i) for i in idxs])
    # multiple of 128 so every chunk splits into full 128-token t-blocks
    # (DoubleRow ldweights rejects partial stationary tiles)
    cpad = max(CS, int(-(-counts.max() // 128) * 128))

    if cpad not in _cache:
        _cache[cpad] = _build(cpad)
    nc = _cache[cpad]

    ntb_r = sum(-(-cs // 128) for _, cs in _chunks(cpad))

    # ---- shared-expert packing (replicated) ----
    sw1s = sw1.reshape(FS, D).T * WS                      # [D, FS]
    s1h, s1l = _split8(sw1s)
    sw1h_p = _pack_k_blocks(s1h, np8)
    sw1l_p = _pack_k_blocks(s1l, np8)
    sw2t = sw2.transpose(0, 2, 1).reshape(FS, D)          # [FS, D]
    sw2_p = _pack_k_blocks(sw2t, np16)

    in_maps = []
    for c in range(N_CORES):
        idx = idxs[c]
        ce = len(idx)
        # routed tokens, fp8, chunk-major
        xgT = np.zeros((D, cpad), np.float32)
        xgT[:, :ce] = t[idx].T
        xg_p = _pack_chunked(xgT.astype(np8), cpad, np8)
        # combine weights / WS as per-partition scalars, t-block cols
        cw_col = np.zeros(ntb_r * 128, np.float32)
        cw_col[:ce] = probs[idx, c] / WS
        # account for chunk-major t-block layout: blocks follow token order
        cwp = np.zeros((128, ntb_r), np.float32)
        col = 0
        pos = 0
        for c0, cs in _chunks(cpad):
            ntb = -(-cs // 128)
            for tb in range(ntb):
                tbs = min(128, cs - tb * 128)
                cwp[:tbs, col] = cw_col[pos:pos + tbs]
                pos += tbs
                col += 1
        # routed weights: hi-only, scaled
        r1h = (rw1[c].T * WS).astype(np8)                 # [D, F]
        rw1_p = _pack_k_blocks(r1h, np8)
        r2h = (rw2[c].T * WS).astype(np8)                 # [F, D] (w2T)
        rw2_p = np.ascontiguousarray(
            r2h.reshape(PR, 2, 128, D).transpose(2, 0, 1, 3)
            .reshape(128, -1))
        # shared tokens hi/lo
        xsT = t[c * TS:(c + 1) * TS].T                    # [D, TS]
        xh = xsT.astype(np8)
        xl = (xsT - xh.astype(np.float32)).astype(np8)
        in_maps.append({
            "xg": xg_p,
            "cwp": cwp,
            "rw1": rw1_p,
            "rw2": rw2_p,
            "xsh": _pack_chunked(xh, TS, np8),
            "xsl": _pack_chunked(xl, TS, np8),
            "sw1h": sw1h_p,
            "sw1l": sw1l_p,
            "sw2": sw2_p,
        })

    res = run_bass_kernel_spmd(nc, in_maps, core_ids=list(range(N_CORES)))

    # ---- combine on host ----
    out = np.empty((T, D), np.float32)
    for c in range(N_CORES):
        out[c * TS:(c + 1) * TS] = res.results[c]["ys"].astype(np.float32)
    for c in range(N_CORES):
        idx = idxs[c]
        out[idx] += res.results[c]["yr"][:len(idx)].astype(np.float32)

    # output biases (zero in the spec, handled exactly anyway)
    if sb2.any() or rb2.any():
        cw = np.zeros((T, E), np.float32)
        np.add.at(cw, (np.arange(T)[:, None], top_i),
                  np.take_along_axis(probs, top_i, axis=1))
        out += sb2.sum(axis=0)[None, :] + cw @ rb2

    return out.reshape(B, S, D)


# revision 10
# speedup vs baseline: 1.5189x; 1.0156x over previous
"""DeepseekMoE kernel for 8 Trainium2 NeuronCores.

Strategy (expert-parallel routed + data-parallel shared, fp8 DoubleRow):
  - Host computes the router (gate matmul, softmax, top-2) in numpy and
    gathers each expert's tokens (MoE dispatch as part of sharding).
  - Core c runs routed expert c's FFN over its gathered tokens; shared
    experts are replicated and each core runs them over a distinct
    512-token slice of the batch.
  - All heavy matmuls use fp8(e4m3) in DoubleRow perf mode: each
    instruction contracts 2 k-tiles (256 rows) at 0.5 cycles per output
    column -- 4x the fp16 rate on the PE.
  - Accuracy: weights are pre-scaled by 2^11 so their hi/lo fp8 splits
    stay out of e4m3's subnormal floor; the descale folds into the ACT
    input scale (layer 1) and the host-side combine weights (layer 2).
      * shared experts (~98% of output norm): layer 1 runs a 3-matmul
        hi/lo compensation (w_hi*x_hi + w_lo*x_hi + w_hi*x_lo), h is
        kept in fp16 and layer 2 runs as a plain fp16 matmul.
      * routed experts (~21% of output norm): plain fp8 both layers.
    Measured end-to-end rel err ~1.1% vs the 2e-2 gate.
  - Layer 2 keeps tokens on the output partition dim (stationary = h,
    moving = w2), so each 128-token block accumulates into exactly one
    PSUM bank, the combine weight is a per-partition tensor_scalar
    multiply, and outputs land in [token, D] layout (no host transpose).
  - Routed and shared f-tile steps are interleaved (Bresenham weave) so
    the ACT-heavy routed phase overlaps the PE-heavy shared phase, with
    a 2-step software pipeline between layer 1 and layer 2.
  - 256-token chunks keep both phases' PSUM footprint at 8 banks total
    (2x p1 double-buffer + 2 token-block accumulators per phase).
"""

import numpy as np
import ml_dtypes

import concourse.bass as bass
import concourse.tile as tile
import concourse.mybir as mybir
from concourse import bacc
from concourse.bass_utils import run_bass_kernel_spmd

B, S, D, F, E, NS, K = 2, 2048, 512, 2048, 8, 2, 2
T = B * S
N_CORES = 8
TS = T // N_CORES          # shared-expert tokens per core
FS = NS * F                # concatenated shared FFN width
CS = 256                   # token chunk (1 PSUM bank per 2-f-tile p1 pair)
KD = D // 128              # 4 k-tiles over D
FR = F // 128              # 16 f-tiles routed
FSH = FS // 128            # 32 f-tiles shared
PR = FR // 2               # 8 routed f-pairs
PS = FSH // 2              # 16 shared f-pairs
WS = 2048.0                # 2^11 weight pre-scale (exact power of two)

F8 = mybir.dt.float8e4
F16 = mybir.dt.float16
F32 = mybir.dt.float32
np8 = ml_dtypes.float8_e4m3
np16 = np.float16

_GELU = mybir.ActivationFunctionType.Gelu
_DR = mybir.MatmulPerfMode.DoubleRow

_cache: dict = {}


def _chunks(total):
    """(start, size) chunk list: CS-sized chunks plus a ragged tail."""
    out, c0 = [], 0
    while c0 < total:
        cs = min(CS, total - c0)
        out.append((c0, cs))
        c0 += cs
    return out


def _weave(ra, sb):
    """Evenly interleave two lists (Bresenham pacing)."""
    out, i, j = [], 0, 0
    while i < len(ra) or j < len(sb):
        if j >= len(sb) or (i < len(ra) and i * len(sb) <= j * len(ra)):
            out.append(ra[i]); i += 1
        else:
            out.append(sb[j]); j += 1
    return out


def _build(cpad: int):
    nc = bacc.Bacc("TRN2", debug=False)

    rchunks = _chunks(cpad)
    schunks = _chunks(TS)
    ntb_r = sum(-(-cs // 128) for _, cs in rchunks)

    xg = nc.dram_tensor("xg", [128, KD * cpad], F8, kind="ExternalInput")
    cwp = nc.dram_tensor("cwp", [128, ntb_r], F32, kind="ExternalInput")
    rw1 = nc.dram_tensor("rw1", [128, KD * F], F8, kind="ExternalInput")
    rw2 = nc.dram_tensor("rw2", [128, F * KD], F8, kind="ExternalInput")
    xsh = nc.dram_tensor("xsh", [128, KD * TS], F8, kind="ExternalInput")
    xsl = nc.dram_tensor("xsl", [128, KD * TS], F8, kind="ExternalInput")
    sw1h = nc.dram_tensor("sw1h", [128, KD * FS], F8, kind="ExternalInput")
    sw1l = nc.dram_tensor("sw1l", [128, KD * FS], F8, kind="ExternalInput")
    sw2 = nc.dram_tensor("sw2", [128, FSH * D], F16, kind="ExternalInput")
    yr = nc.dram_tensor("yr", [cpad, D], F16, kind="ExternalOutput")
    ys = nc.dram_tensor("ys", [TS, D], F16, kind="ExternalOutput")

    with tile.TileContext(nc) as tc:
        with (
            tc.tile_pool(name="wts", bufs=1) as wts,
            tc.tile_pool(name="acts", bufs=1) as acts,
            tc.tile_pool(name="hpr", bufs=4) as hpr,
            tc.tile_pool(name="hps", bufs=4) as hps,
            tc.tile_pool(name="op", bufs=3) as op,
            tc.tile_pool(name="psr", bufs=2, space="PSUM") as psr,
            tc.tile_pool(name="pss", bufs=2, space="PSUM") as pss,
            tc.tile_pool(name="por", bufs=1, space="PSUM") as por,
            tc.tile_pool(name="pos", bufs=1, space="PSUM") as pos,
        ):
            # ---- warmup while the first DMAs are in flight: trigger the
            # GELU table load and ramp the PE p-state ----
            warm = wts.tile([128, 256], F8, name="warm_in")
            nc.vector.memset(warm[:], 0.0)
            wh = hpr.tile([128, 256], F8, name="warm_h")
            nc.scalar.activation(wh[:], warm[:, 0:256], _GELU, bias=0.0)
            wp = psr.tile([128, 512], F32, tag="p1", name="warm_p")
            w3 = warm.rearrange("p (two n) -> p two n", two=2)
            for _ in range(8):
                nc.tensor.matmul(wp[:, 0:128], w3[:, :, 0:128], w3[:],
                                 start=True, stop=True, perf_mode=_DR)

            # ---- resident SBUF images ----
            xg_sb = acts.tile([128, KD * cpad], F8, name="xg_sb")
            cwp_sb = acts.tile([128, ntb_r], F32, name="cwp_sb")
            rw1_sb = wts.tile([128, KD * F], F8, name="rw1_sb")
            rw2_sb = wts.tile([128, F * KD], F8, name="rw2_sb")
            xsh_sb = acts.tile([128, KD * TS], F8, name="xsh_sb")
            xsl_sb = acts.tile([128, KD * TS], F8, name="xsl_sb")
            sw1h_sb = wts.tile([128, KD * FS], F8, name="sw1h_sb")
            sw1l_sb = wts.tile([128, KD * FS], F8, name="sw1l_sb")
            sw2_sb = wts.tile([128, FSH * D], F16, name="sw2_sb")

            # 3-d views for DoubleRow operand slicing
            rw1_v = rw1_sb.rearrange("p (k f) -> p k f", k=KD)
            rw2_v = rw2_sb.rearrange("p (j two d) -> p j two d", j=PR, two=2)
            sw1h_v = sw1h_sb.rearrange("p (k f) -> p k f", k=KD)
            sw1l_v = sw1l_sb.rearrange("p (k f) -> p k f", k=KD)
            sw2_v = sw2_sb.rearrange("p (f d) -> p f d", f=FSH)
            xg_v = xg_sb  # chunk-major; sliced via offsets below
            xoff = [0]
            for _, cs in rchunks:
                xoff.append(xoff[-1] + KD * cs)
            soff = [0]
            for _, cs in schunks:
                soff.append(soff[-1] + KD * cs)

            def w1_group_dma(dst, src, f_lo, f_hi, n_f):
                d3 = dst.rearrange("p (k f) -> p k f", k=KD)
                s3 = src.ap().rearrange("p (k f) -> p k f", k=KD)
                nc.sync.dma_start(d3[:, :, f_lo * 128:f_hi * 128],
                                  s3[:, :, f_lo * 128:f_hi * 128])

            # ---- consumption-ordered preload: small first groups so the
            # first R and S pair-steps unblock ASAP, then stream the rest ----
            w1_group_dma(rw1_sb, rw1, 0, 2, FR)
            nc.gpsimd.dma_start(xg_sb[:, 0:xoff[1]], xg.ap()[:, 0:xoff[1]])
            w1_group_dma(sw1h_sb, sw1h, 0, 2, FSH)
            w1_group_dma(sw1l_sb, sw1l, 0, 2, FSH)
            nc.gpsimd.dma_start(xsh_sb[:], xsh.ap())
            nc.gpsimd.dma_start(xsl_sb[:], xsl.ap())
            w1_group_dma(rw1_sb, rw1, 2, 4, FR)
            nc.sync.dma_start(rw2_sb[:, 0:2 * 2 * D], rw2.ap()[:, 0:2 * 2 * D])
            w1_group_dma(sw1h_sb, sw1h, 2, 6, FSH)
            w1_group_dma(sw1l_sb, sw1l, 2, 6, FSH)
            nc.sync.dma_start(sw2_sb[:, 0:4 * D], sw2.ap()[:, 0:4 * D])
            w1_group_dma(rw1_sb, rw1, 4, 8, FR)
            nc.sync.dma_start(cwp_sb[:], cwp.ap())
            w1_group_dma(sw1h_sb, sw1h, 6, 10, FSH)
            w1_group_dma(sw1l_sb, sw1l, 6, 10, FSH)
            nc.sync.dma_start(sw2_sb[:, 4 * D:8 * D], sw2.ap()[:, 4 * D:8 * D])
            nc.sync.dma_start(rw2_sb[:, 2 * 2 * D:5 * 2 * D],
                              rw2.ap()[:, 2 * 2 * D:5 * 2 * D])
            if len(rchunks) > 1:
                nc.gpsimd.dma_start(xg_sb[:, xoff[1]:xoff[2]],
                                    xg.ap()[:, xoff[1]:xoff[2]])
            w1_group_dma(rw1_sb, rw1, 8, 12, FR)
            w1_group_dma(sw1h_sb, sw1h, 10, 16, FSH)
            w1_group_dma(sw1l_sb, sw1l, 10, 16, FSH)
            nc.sync.dma_start(sw2_sb[:, 8 * D:14 * D], sw2.ap()[:, 8 * D:14 * D])
            nc.sync.dma_start(rw2_sb[:, 5 * 2 * D:], rw2.ap()[:, 5 * 2 * D:])
            if len(rchunks) > 2:
                nc.gpsimd.dma_start(xg_sb[:, xoff[2]:xoff[3]],
                                    xg.ap()[:, xoff[2]:xoff[3]])
            w1_group_dma(rw1_sb, rw1, 12, 16, FR)
            w1_group_dma(sw1h_sb, sw1h, 16, 22, FSH)
            w1_group_dma(sw1l_sb, sw1l, 16, 22, FSH)
            nc.sync.dma_start(sw2_sb[:, 14 * D:20 * D], sw2.ap()[:, 14 * D:20 * D])
            if len(rchunks) > 3:
                nc.gpsimd.dma_start(xg_sb[:, xoff[3]:], xg.ap()[:, xoff[3]:])
            w1_group_dma(sw1h_sb, sw1h, 22, 28, FSH)
            w1_group_dma(sw1l_sb, sw1l, 22, 28, FSH)
            nc.sync.dma_start(sw2_sb[:, 20 * D:26 * D], sw2.ap()[:, 20 * D:26 * D])
            w1_group_dma(sw1h_sb, sw1h, 28, 32, FSH)
            w1_group_dma(sw1l_sb, sw1l, 28, 32, FSH)
            nc.sync.dma_start(sw2_sb[:, 26 * D:], sw2.ap()[:, 26 * D:])

            # ---- step list: (phase, chunk_idx, pair_idx), woven ----
            rsteps = [("R", ci, j) for ci in range(len(rchunks))
                      for j in range(PR)]
            ssteps = [("S", ci, j) for ci in range(len(schunks))
                      for j in range(PS)]
            # R pairs are ready first (small x/w1 groups); give them a head
            # start so the PE has work while the shared weights stream in
            head = min(3, len(rsteps))
            steps = rsteps[:head] + _weave(rsteps[head:], ssteps)

            # per-chunk global t-block column base for cwp
            tb_base = []
            acc = 0
            for _, cs in rchunks:
                tb_base.append(acc)
                acc += -(-cs // 128)

            h_tiles: dict = {}
            po_r: list = [None]
            po_s: list = [None]

            def stage_a(step):
                ph, ci, j = step
                if ph == "R":
                    c0, cs = rchunks[ci]
                    p1 = psr.tile([128, 2 * cs], F32, tag="p1", name="p1r")
                    for i01 in range(2):
                        f = 2 * j + i01
                        dst = p1[:, i01 * cs:(i01 + 1) * cs]
                        xo = xoff[ci]
                        for b in range(2):
                            nc.tensor.matmul(
                                dst,
                                rw1_v[:, 2 * b:2 * b + 2, f * 128:(f + 1) * 128],
                                xg_v[:, xo:xo + KD * cs].rearrange(
                                    "p (k c) -> p k c", k=KD)[:, 2 * b:2 * b + 2, :],
                                start=(b == 0), stop=(b == 1), perf_mode=_DR)
                    h = hpr.tile([128, 2 * cs], F8, name="hr")
                    nc.scalar.activation(h[:], p1[:], _GELU, bias=0.0,
                                         scale=1.0 / WS)
                else:
                    c0, cs = schunks[ci]
                    p1 = pss.tile([128, 2 * cs], F32, tag="p1", name="p1s")
                    xh3 = xsh_sb[:, soff[ci]:soff[ci] + KD * cs].rearrange(
                        "p (k c) -> p k c", k=KD)
                    xl3 = xsl_sb[:, soff[ci]:soff[ci] + KD * cs].rearrange(
                        "p (k c) -> p k c", k=KD)
                    for i01 in range(2):
                        f = 2 * j + i01
                        dst = p1[:, i01 * cs:(i01 + 1) * cs]
                        fs = slice(f * 128, (f + 1) * 128)
                        for b in range(2):
                            ks = slice(2 * b, 2 * b + 2)
                            nc.tensor.matmul(
                                dst, sw1h_v[:, ks, fs], xh3[:, ks, :],
                                start=(b == 0), stop=False, perf_mode=_DR)
                            nc.tensor.matmul(
                                dst, sw1l_v[:, ks, fs], xh3[:, ks, :],
                                start=False, stop=False, perf_mode=_DR)
                            nc.tensor.matmul(
                                dst, sw1h_v[:, ks, fs], xl3[:, ks, :],
                                start=False, stop=(b == 1), perf_mode=_DR)
                    h = hps.tile([128, 2 * cs], F16, name="hs")
                    nc.scalar.activation(h[:], p1[:], _GELU, bias=0.0,
                                         scale=1.0 / WS)
                return h

            def stage_b(step, h, is_last_step):
                ph, ci, j = step
                if ph == "R":
                    c0, cs = rchunks[ci]
                    ntb = -(-cs // 128)
                    if j == 0:
                        po_r[0] = [por.tile([128, D], F32, tag=f"r{tb}",
                                            name=f"por{tb}")
                                   for tb in range(ntb)]
                    h3 = h.rearrange("p (two c) -> p two c", two=2)
                    for tb in range(ntb):
                        tbs = min(128, cs - tb * 128)
                        nc.tensor.matmul(
                            po_r[0][tb][0:tbs, :],
                            h3[:, :, tb * 128:tb * 128 + tbs],
                            rw2_v[:, j, :, :],
                            start=(j == 0), stop=(j == PR - 1), perf_mode=_DR)
                    if j == PR - 1:
                        # per-t-block evac + DMA so the store of tb0 overlaps
                        # the evac of tb1 (pipelined drain)
                        o = op.tile([128, ntb * D], F16, name="or")
                        for tb in range(ntb):
                            tbs = min(128, cs - tb * 128)
                            nc.vector.tensor_scalar_mul(
                                o[0:tbs, tb * D:(tb + 1) * D],
                                po_r[0][tb][0:tbs, :],
                                cwp_sb[0:tbs, tb_base[ci] + tb:
                                       tb_base[ci] + tb + 1])
                            dma = nc.sync.dma_start if is_last_step \
                                else nc.gpsimd.dma_start
                            dma(yr.ap()[c0 + tb * 128:c0 + tb * 128 + tbs, :],
                                o[0:tbs, tb * D:(tb + 1) * D])
                else:
                    c0, cs = schunks[ci]
                    ntb = -(-cs // 128)
                    if j == 0:
                        po_s[0] = [pos.tile([128, D], F32, tag=f"s{tb}",
                                            name=f"pos{tb}")
                                   for tb in range(ntb)]
                    for i01 in range(2):
                        f = 2 * j + i01
                        for tb in range(ntb):
                            tbs = min(128, cs - tb * 128)
                            nc.tensor.matmul(
                                po_s[0][tb][0:tbs, :],
                                h[:, i01 * cs + tb * 128:
                                  i01 * cs + tb * 128 + tbs],
                                sw2_v[:, f, :],
                                start=(f == 0), stop=(f == FSH - 1))
                    if j == PS - 1:
                        # evacs split ACT/DVE so the two t-blocks drain in
                        # parallel, each followed by its own store
                        o = op.tile([128, ntb * D], F16, name="os")
                        for tb in range(ntb):
                            tbs = min(128, cs - tb * 128)
                            if tb % 2 == 1:
                                nc.scalar.copy(o[0:tbs, tb * D:(tb + 1) * D],
                                               po_s[0][tb][0:tbs, :])
                            else:
                                nc.vector.tensor_copy(
                                    o[0:tbs, tb * D:(tb + 1) * D],
                                    po_s[0][tb][0:tbs, :])
                            dma = nc.sync.dma_start if is_last_step \
                                else nc.gpsimd.dma_start
                            dma(ys.ap()[c0 + tb * 128:c0 + tb * 128 + tbs, :],
                                o[0:tbs, tb * D:(tb + 1) * D])

            LOOKAHEAD = 2
            for i in range(len(steps) + LOOKAHEAD):
                if i < len(steps):
                    h_tiles[i] = stage_a(steps[i])
                jj = i - LOOKAHEAD
                if jj >= 0:
                    stage_b(steps[jj], h_tiles.pop(jj), jj == len(steps) - 1)

    nc.compile()
    return nc


def _pack_k_blocks(a2d, dtype):
    """[K*128, N] -> [128, K*N] with k-blocks along the free dim."""
    k = a2d.shape[0] // 128
    return np.ascontiguousarray(
        a2d.reshape(k, 128, -1).transpose(1, 0, 2).reshape(128, -1)
        .astype(dtype))


def _pack_chunked(xT, total, dtype):
    """[D, total] -> [128, KD*total] chunk-major k-blocks."""
    parts = []
    for c0, cs in _chunks(total):
        blk = xT[:, c0:c0 + cs]
        parts.append(blk.reshape(KD, 128, cs).transpose(1, 0, 2)
                     .reshape(128, -1))
    return np.ascontiguousarray(np.concatenate(parts, axis=1).astype(dtype))


def _split8(a):
    """Scaled hi/lo e4m3 split of an array (applied at scale WS)."""
    hi = a.astype(np8)
    lo = (a - hi.astype(np.float32)).astype(np8)
    return hi, lo


def _numpy_fallback(x, gate_w, gate_b, sw1, sb1, sw2, sb2, rw1, rb1, rw2, rb2):
    from scipy.special import erf
    t = x.reshape(-1, D)
    gelu = lambda u: 0.5 * u * (1.0 + erf(u / np.sqrt(2.0)))
    hs = gelu(np.einsum('td,nfd->ntf', t, sw1) + sb1[:, None, :])
    shared = np.einsum('ntf,ndf->td', hs, sw2) + sb2.sum(axis=0)
    logits = t @ gate_w.T + gate_b
    m = logits.max(axis=1, keepdims=True)
    ex = np.exp(logits - m)
    probs = ex / ex.sum(axis=1, keepdims=True)
    top_i = np.argpartition(-probs, K - 1, axis=1)[:, :K]
    cw = np.zeros_like(probs)
    np.add.at(cw, (np.arange(t.shape[0])[:, None], top_i),
              np.take_along_axis(probs, top_i, axis=1))
    hr = gelu(np.einsum('td,efd->etf', t, rw1) + rb1[:, None, :])
    oe = np.einsum('etf,edf->etd', hr, rw2) + rb2[:, None, :]
    routed = np.einsum('etd,te->td', oe, cw)
    return (shared + routed).reshape(x.shape).astype(np.float32)


def kernel(x, gate_w, gate_b, sw1, sb1, sw2, sb2, rw1, rb1, rw2, rb2):
    x = np.asarray(x, np.float32)
    gate_w = np.asarray(gate_w, np.float32)
    gate_b = np.asarray(gate_b, np.float32)
    sw1 = np.asarray(sw1, np.float32)
    sb1 = np.asarray(sb1, np.float32)
    sw2 = np.asarray(sw2, np.float32)
    sb2 = np.asarray(sb2, np.float32)
    rw1 = np.asarray(rw1, np.float32)
    rb1 = np.asarray(rb1, np.float32)
    rw2 = np.asarray(rw2, np.float32)
    rb2 = np.asarray(rb2, np.float32)

    if sb1.any() or rb1.any():
        # device path folds first-layer biases away (they are zero in the
        # problem spec); fall back to exact numpy if that ever changes
        return _numpy_fallback(x, gate_w, gate_b, sw1, sb1, sw2, sb2,
                               rw1, rb1, rw2, rb2)

    t = x.reshape(T, D)

    # ---- router on host (dispatch/sharding step) ----
    logits = t @ gate_w.T + gate_b
    m = logits.max(axis=1, keepdims=True)
    ex = np.exp(logits - m)
    probs = ex / ex.sum(axis=1, keepdims=True)
    top_i = np.argpartition(-probs, K - 1, axis=1)[:, :K]

    sel = np.zeros((T, E), bool)
    sel[np.arange(T)[:, None], top_i] = True
    idxs = [np.nonzero(sel[:, e])[0] for e in range(E)]
    counts = np.array([len(i) for i in idxs])
    # multiple of 128 so every chunk splits into full 128-token t-blocks
    # (DoubleRow ldweights rejects partial stationary tiles)
    cpad = max(CS, int(-(-counts.max() // 128) * 128))

    if cpad not in _cache:
        _cache[cpad] = _build(cpad)
    nc = _cache[cpad]

    ntb_r = sum(-(-cs // 128) for _, cs in _chunks(cpad))

    # ---- shared-expert packing (replicated) ----
    sw1s = sw1.reshape(FS, D).T * WS                      # [D, FS]
    s1h, s1l = _split8(sw1s)
    sw1h_p = _pack_k_blocks(s1h, np8)
    sw1l_p = _pack_k_blocks(s1l, np8)
    sw2t = sw2.transpose(0, 2, 1).reshape(FS, D)          # [FS, D]
    sw2_p = _pack_k_blocks(sw2t, np16)

    in_maps = []
    for c in range(N_CORES):
        idx = idxs[c]
        ce = len(idx)
        # routed tokens, fp8, chunk-major
        xgT = np.zeros((D, cpad), np.float32)
        xgT[:, :ce] = t[idx].T
        xg_p = _pack_chunked(xgT.astype(np8), cpad, np8)
        # combine weights / WS as per-partition scalars, t-block cols
        cw_col = np.zeros(ntb_r * 128, np.float32)
        cw_col[:ce] = probs[idx, c] / WS
        # account for chunk-major t-block layout: blocks follow token order
        cwp = np.zeros((128, ntb_r), np.float32)
        col = 0
        pos = 0
        for c0, cs in _chunks(cpad):
            ntb = -(-cs // 128)
            for tb in range(ntb):
                tbs = min(128, cs - tb * 128)
                cwp[:tbs, col] = cw_col[pos:pos + tbs]
                pos += tbs
                col += 1
        # routed weights: hi-only, scaled
        r1h = (rw1[c].T * WS).astype(np8)                 # [D, F]
        rw1_p = _pack_k_blocks(r1h, np8)
        r2h = (rw2[c].T * WS).astype(np8)                 # [F, D] (w2T)
        rw2_p = np.ascontiguousarray(
            r2h.reshape(PR, 2, 128, D).transpose(2, 0, 1, 3)
            .reshape(128, -1))
        # shared tokens hi/lo
        xsT = t[c * TS:(c + 1) * TS].T                    # [D, TS]
        xh = xsT.astype(np8)
        xl = (xsT - xh.astype(np.float32)).astype(np8)
        in_maps.append({
            "xg": xg_p,
            "cwp": cwp,
            "rw1": rw1_p,
            "rw2": rw2_p,
            "xsh": _pack_chunked(xh, TS, np8),
            "xsl": _pack_chunked(xl, TS, np8),
            "sw1h": sw1h_p,
            "sw1l": sw1l_p,
            "sw2": sw2_p,
        })

    res = run_bass_kernel_spmd(nc, in_maps, core_ids=list(range(N_CORES)))

    # ---- combine on host ----
    out = np.empty((T, D), np.float32)
    for c in range(N_CORES):
        out[c * TS:(c + 1) * TS] = res.results[c]["ys"].astype(np.float32)
    for c in range(N_CORES):
        idx = idxs[c]
        out[idx] += res.results[c]["yr"][:len(idx)].astype(np.float32)

    # output biases (zero in the spec, handled exactly anyway)
    if sb2.any() or rb2.any():
        cw = np.zeros((T, E), np.float32)
        np.add.at(cw, (np.arange(T)[:, None], top_i),
                  np.take_along_axis(probs, top_i, axis=1))
        out += sb2.sum(axis=0)[None, :] + cw @ rb2

    return out.reshape(B, S, D)
